# revision 30
# baseline (speedup 1.0000x reference)
"""Trainium2 Bass kernel for a NeRF-style mip ray marcher (alpha compositing).

Math (per ray, S=48 samples, C=32 channels):
    dd_i     = softplus(0.5*(den_i+den_{i+1}) - 1) * (dep_{i+1} - dep_i)   i=0..46
    F_k      = exp(-sum_{j<k} dd_j)              k=0..47   (F_0 = 1)
    w_i      = F_i - F_{i+1}                     (compositing weights)
    rgb_c    = sum_s G_s * col_{s,c} - 1         where G_s = F_{s-1} - F_{s+1}
               (G_s = w_{s-1}+w_s = 2*midpoint weight; folds the *2-1 output map)
    depth    = clip(0.5 * (sum_s G_s dep_s) / (1 - F_47), dmin, dmax)

Sharding: 65536 rays split evenly across 8 NeuronCores (SPMD, no comms).
"""

import sys

sys.path.insert(0, "/opt/trn_rl_repo")

import numpy as np

import concourse.bass as bass
import concourse.bacc as bacc
import concourse.mybir as mybir
import concourse.tile as tile
from concourse.bass_utils import run_bass_kernel_spmd

B, R, S, C = 4, 16384, 48, 32
NCORES = 8
NRAYS = B * R
PER = NRAYS // NCORES      # 8192 rays per core
T = 4                      # rays per partition per tile
RPT = 128 * T              # 512 rays per tile
NT = PER // RPT            # 16 tiles per core

F32 = mybir.dt.float32
ALU = mybir.AluOpType
ACTF = mybir.ActivationFunctionType

_CACHE = {}


def _pin_act_tables():
    """Restrict Exp/Ln/Copy to the one table set containing all three, so the
    table-load placement pass never alternates sets (each switch costs ~2.7us
    on hardware). Set ids are positional, so membership is edited in place
    rather than filtering entries."""
    import concourse.bacc as _bacc

    orig = _bacc.get_activation_tables
    keep = {ACTF.Exp, ACTF.Ln}

    def patched(arch):
        tabs = orig(arch)
        for name, s in tabs.items():
            if name != "natural_log_exp_and_others":
                s -= keep
        return tabs

    _bacc.get_activation_tables = patched


_pin_act_tables()

# op-placement / tiling configuration (tuned via timeline-sim sweeps)
CFG = {
    "T": 4,        # rays per partition per tile
    "small": "gpsimd",   # engine for small per-ray elementwise ops
    "scan": "vector",    # engine for the cumsum scan
    "tree": 0,     # levels of gpsimd pair-add reduction on wc before DVE reduce
    "cbufs": 3,    # colors pool buffers
    "wcbufs": 2,   # wc pool buffers
    "rgb": "pe",   # "dve": strided tensor_reduce; "pe": transpose+comb matmul
    "ndvecopy": 2, # with rgb=pe: how many of the 12 PSUM->SBUF copies go to DVE
    "trdtype": "f32",  # transpose input dtype view: "f32" or "f32r"
    "NG": 1,       # sample-windows per PSUM->SBUF copy group
    "trbufs": 4,   # PSUM transpose-staging buffers
    "smbufs": 4,   # small pool buffers
    "dbatch": 4,   # tiles per dens/deps input DMA and depth-out DMA
    "multsplit": 2,  # split wc multiply into this many DVE ops
    "outbufs": 4,  # output pool buffers
    "mmswap": 0,   # 1: wcT stationary, comb moving -> rgb lands in ray layout
}


def _build_module(reps=1, cfg=CFG):
    global T, RPT, NT
    T = cfg["T"]
    RPT = 128 * T
    NT = PER // RPT
    nc = bacc.Bacc("TRN2", target_bir_lowering=False, debug=False, num_devices=NCORES)

    colors = nc.dram_tensor("colors", [PER, S, C], F32, kind="ExternalInput").ap()
    dens = nc.dram_tensor("densities", [PER, S], F32, kind="ExternalInput").ap()
    deps = nc.dram_tensor("depths", [PER, S], F32, kind="ExternalInput").ap()
    clipb = nc.dram_tensor("clipb", [128, 2], F32, kind="ExternalInput").ap()
    identb = nc.dram_tensor("identb", [128, 128], F32, kind="ExternalInput").ap()
    combb = nc.dram_tensor("combb", [128, C], mybir.dt.float32r, kind="ExternalInput").ap()

    orgb = nc.dram_tensor("orgb", [PER, C], F32, kind="ExternalOutput").ap()
    odep = nc.dram_tensor("odep", [PER, 1], F32, kind="ExternalOutput").ap()
    ow = nc.dram_tensor("ow", [PER, S - 1], F32, kind="ExternalOutput").ap()

    smallE = getattr(nc, cfg["small"])
    scanE = getattr(nc, cfg["scan"])

    with tile.TileContext(nc) as tc:
        with (
            tc.tile_pool(name="const", bufs=1) as constp,
            tc.tile_pool(name="colors", bufs=cfg["cbufs"]) as cpool,
            tc.tile_pool(name="wc", bufs=cfg["wcbufs"]) as wcpool,
            tc.tile_pool(name="sm", bufs=cfg["smbufs"]) as sm,
            tc.tile_pool(name="outp", bufs=cfg["outbufs"]) as outp,
            tc.tile_pool(name="pstr", bufs=cfg["trbufs"], space="PSUM") as pstr,
            tc.tile_pool(name="psacc", bufs=2, space="PSUM") as psacc,
            tc.tile_pool(name="pstrb", bufs=2, space="PSUM") as pstrb,
        ):
            # constants
            clipt = constp.tile([128, 2], F32)
            nc.sync.dma_start(clipt[:], clipb)
            dmin = clipt[:, 0:1]
            dmax = clipt[:, 1:2]

            mask = constp.tile([128, T * S], F32)
            nc.gpsimd.memset(mask[:], 1.0)
            nc.gpsimd.memset(
                mask[:].rearrange("p (t s) -> p t s", t=T)[:, :, 0:1], 0.0
            )

            neg1 = constp.tile([128, 1], F32)
            nc.gpsimd.memset(neg1[:], -1.0)

            if cfg["rgb"] == "pe":
                ident = constp.tile([128, 128], F32)
                nc.sync.dma_start(ident[:], identb)
                comb = constp.tile([128, C], mybir.dt.float32r)
                nc.sync.dma_start(comb[:], combb)

            for i in range(NT * reps):
                i = i % NT
                r0 = i * RPT
                rs = slice(r0, r0 + RPT)

                ct = cpool.tile([128, T * S * C], F32)
                ct4 = ct[:].rearrange("p (t s c) -> p t s c", t=T, s=S)
                nc.sync.dma_start(
                    ct4, colors[rs].rearrange("(p t) s c -> p t s c", p=128)
                )

                DB = cfg["dbatch"]
                if i % DB == 0:
                    rb = slice(r0, r0 + RPT * DB)
                    dtb = sm.tile([128, DB * T * S], F32, tag="dens")
                    nc.scalar.dma_start(
                        dtb[:].rearrange("p (j t s) -> p j t s", j=DB, t=T),
                        dens[rb].rearrange("(j p t) s -> p j t s", j=DB, p=128),
                    )
                    ztb = sm.tile([128, DB * T * S], F32, tag="deps")
                    nc.scalar.dma_start(
                        ztb[:].rearrange("p (j t s) -> p j t s", j=DB, t=T),
                        deps[rb].rearrange("(j p t) s -> p j t s", j=DB, p=128),
                    )
                    dob = outp.tile([128, DB * T], F32, tag="dob")
                j = i % DB
                d3 = dtb[:, j * T * S : (j + 1) * T * S].rearrange(
                    "p (t s) -> p t s", t=T
                )
                z3 = ztb[:, j * T * S : (j + 1) * T * S].rearrange(
                    "p (t s) -> p t s", t=T
                )

                # densities midpoint sum (pre-affine) [128, T, 47]
                pd = sm.tile([128, T * (S - 1)], F32, tag="pd")
                pd3 = pd[:].rearrange("p (t s) -> p t s", t=T)
                smallE.tensor_add(pd3, d3[:, :, 0 : S - 1], d3[:, :, 1:S])

                # dm = softplus(0.5*pd - 1) = ln(1 + exp(0.5*pd - 1))
                es = sm.tile([128, T * (S - 1)], F32, tag="es")
                es3 = es[:].rearrange("p (t s) -> p t s", t=T)
                nc.scalar.activation(es3, pd3, ACTF.Exp, bias=neg1[:], scale=0.5)
                dm = sm.tile([128, T * (S - 1)], F32, tag="dm")
                dm3 = dm[:].rearrange("p (t s) -> p t s", t=T)
                nc.scalar.activation(dm3, es3, ACTF.Ln, bias=1.0, scale=1.0)

                # delta = dep[s+1] - dep[s]
                dl = sm.tile([128, T * (S - 1)], F32, tag="delta")
                dl3 = dl[:].rearrange("p (t s) -> p t s", t=T)
                smallE.tensor_sub(dl3, z3[:, :, 1:S], z3[:, :, 0 : S - 1])

                # e-buffer: slot 0 = 0, slots 1..47 = dd
                eb = sm.tile([128, T * S], F32, tag="ebuf")
                e3 = eb[:].rearrange("p (t s) -> p t s", t=T)
                nc.gpsimd.memset(e3[:, :, 0:1], 0.0)
                smallE.tensor_mul(e3[:, :, 1:S], dm3, dl3)

                # exclusive cumsum of dd via masked scan (mask resets at ray start)
                Dt = sm.tile([128, T * S], F32, tag="Dt")
                scanE.tensor_tensor_scan(
                    Dt[:], mask[:], eb[:], 0.0, op0=ALU.mult, op1=ALU.add
                )

                # F = exp(-D)
                Ft = sm.tile([128, T * S], F32, tag="Ft")
                F3 = Ft[:].rearrange("p (t s) -> p t s", t=T)
                nc.scalar.activation(Ft[:], Dt[:], ACTF.Exp, scale=-1.0)

                # weights w_i = F_i - F_{i+1}
                wt = outp.tile([128, T * (S - 1)], F32, tag="w")
                w3 = wt[:].rearrange("p (t s) -> p t s", t=T)
                smallE.tensor_sub(w3, F3[:, :, 0 : S - 1], F3[:, :, 1:S])
                nc.scalar.dma_start(ow[rs].rearrange("(p t) s -> p t s", p=128), w3)

                # G_s = F_{s-1} - F_{s+1}; edges G_0 = w_0, G_47 = w_46
                Gt = sm.tile([128, T * S], F32, tag="Gt")
                G3 = Gt[:].rearrange("p (t s) -> p t s", t=T)
                smallE.tensor_sub(
                    G3[:, :, 1 : S - 1], F3[:, :, 0 : S - 2], F3[:, :, 2:S]
                )
                nc.gpsimd.tensor_copy(G3[:, :, 0:1], w3[:, :, 0:1])
                nc.gpsimd.tensor_copy(G3[:, :, S - 1 : S], w3[:, :, S - 2 : S - 1])

                # wc = colors * G (broadcast over channels)
                wc = wcpool.tile([128, T * S * C], F32)
                wc4 = wc[:].rearrange("p (t s c) -> p t s c", t=T, s=S)
                Gb = (
                    Gt[:]
                    .rearrange("p (t s) -> p t s", t=T)
                    .unsqueeze(3)
                    .broadcast_to((128, T, S, C))
                )
                ms = cfg["multsplit"]
                step = S // ms
                for m in range(ms):
                    sl = slice(m * step, (m + 1) * step)
                    nc.vector.tensor_mul(wc4[:, :, sl, :], ct4[:, :, sl, :], Gb[:, :, sl, :])

                # rgb = sum_s wc - 1
                rgbo = outp.tile([128, T * C], F32, tag="rgbo")
                rgbo3 = rgbo[:].rearrange("p (t c) -> p t c", t=T)
                if cfg["rgb"] == "dve":
                    red_in = wc
                    sred = S
                    for lvl in range(cfg["tree"]):
                        half = sred // 2
                        h = wcpool.tile([128, T * half * C], F32, tag=f"h{lvl}")
                        h4 = h[:].rearrange("p (t s c) -> p t s c", t=T, s=half)
                        r4 = red_in[:].rearrange("p (t s c) -> p t s c", t=T, s=sred)
                        nc.gpsimd.tensor_add(h4, r4[:, :, 0:half, :], r4[:, :, half:sred, :])
                        red_in = h
                        sred = half
                    rgbs = sm.tile([128, T * C], F32, tag="rgbs")
                    rgbs3 = rgbs[:].rearrange("p (t c) -> p t c", t=T)
                    nc.vector.tensor_reduce(
                        rgbs3,
                        red_in[:].rearrange("p (t s c) -> p t c s", t=T, s=sred),
                        axis=mybir.AxisListType.X,
                        op=ALU.add,
                    )
                    nc.scalar.activation(rgbo[:], rgbs[:], ACTF.Copy, bias=-1.0)
                else:
                    # PE path: transpose 128x128 blocks of wc into PSUM so the
                    # sample axis lands on partitions, copy to SBUF, then a
                    # comb matmul (one-hot channel selector) contracts the 4
                    # samples of each block; PSUM-accumulate over the 12
                    # sample-windows. acc[c', t*128+p] = sum_s G*col.
                    NW = S // 4                # 12 sample-windows per t
                    NG = cfg["NG"]             # windows per PSUM copy group
                    f32r = mybir.dt.float32r
                    if cfg["mmswap"]:
                        # t-major: each t accumulates into its own PSUM bank
                        # (pool-rotated), so matmul start=True never clobbers
                        # another t's partial sums. rgb lands in ray layout.
                        for t in range(T):
                            acc = psacc.tile([128, C], F32, tag="acc")
                            for og in range(NW // NG):
                                trp = pstr.tile([128, 128 * NG], F32, tag="tr")
                                for g in range(NG):
                                    o = og * NG + g
                                    blk = wc[
                                        :,
                                        t * S * C + o * 128 : t * S * C + (o + 1) * 128,
                                    ]
                                    idap = ident[:]
                                    if cfg["trdtype"] == "f32r":
                                        blk = blk.bitcast(f32r)
                                        idap = idap.bitcast(f32r)
                                    nc.tensor.transpose(
                                        trp[:, g * 128 : (g + 1) * 128], blk, idap
                                    )
                                wcsb = sm.tile(
                                    [128, 128 * NG], mybir.dt.float32r, tag="wcsb"
                                )
                                if og < cfg["ndvecopy"]:
                                    nc.vector.tensor_copy(wcsb[:], trp[:])
                                else:
                                    nc.scalar.copy(wcsb[:], trp[:])
                                for g in range(NG):
                                    o = og * NG + g
                                    nc.tensor.matmul(
                                        acc[:],
                                        wcsb[:, g * 128 : (g + 1) * 128],
                                        comb[:],
                                        start=(o == 0),
                                        stop=(o == NW - 1),
                                    )
                            nc.scalar.activation(
                                rgbo3[:, t, :], acc[:], ACTF.Copy, bias=-1.0
                            )
                    else:
                        acc = psacc.tile([C, 128 * T], F32, tag="acc")
                        for og in range(NW // NG):
                            trp = pstr.tile([128, 128 * T * NG], F32, tag="tr")
                            for g in range(NG):
                                o = og * NG + g
                                for t in range(T):
                                    blk = wc[
                                        :,
                                        t * S * C + o * 128 : t * S * C + (o + 1) * 128,
                                    ]
                                    idap = ident[:]
                                    if cfg["trdtype"] == "f32r":
                                        blk = blk.bitcast(f32r)
                                        idap = idap.bitcast(f32r)
                                    nc.tensor.transpose(
                                        trp[:, (g * T + t) * 128 : (g * T + t + 1) * 128],
                                        blk,
                                        idap,
                                    )
                            wcsb = sm.tile(
                                [128, 128 * T * NG], mybir.dt.float32r, tag="wcsb"
                            )
                            if og < cfg["ndvecopy"]:
                                nc.vector.tensor_copy(wcsb[:], trp[:])
                            else:
                                nc.scalar.copy(wcsb[:], trp[:])
                            for g in range(NG):
                                o = og * NG + g
                                nc.tensor.matmul(
                                    acc[:],
                                    comb[:],
                                    wcsb[:, g * 128 * T : (g + 1) * 128 * T],
                                    start=(o == 0),
                                    stop=(o == NW - 1),
                                )
                        accsb = sm.tile([C, 128 * T], F32, tag="accs")
                        nc.scalar.copy(accsb[:], acc[:])
                        for t in range(T):
                            trb = pstrb.tile([128, C], F32, tag="trb")
                            nc.tensor.transpose(
                                trb[:], accsb[:, t * 128 : (t + 1) * 128], ident[:C, :C]
                            )
                            nc.scalar.activation(
                                rgbo3[:, t, :], trb[:], ACTF.Copy, bias=-1.0
                            )
                nc.sync.dma_start(
                    orgb[rs].rearrange("(p t) c -> p t c", p=128), rgbo3
                )

                # depth numerator: sum_s G_s * dep_s
                zw = sm.tile([128, T * S], F32, tag="zw")
                zw3 = zw[:].rearrange("p (t s) -> p t s", t=T)
                smallE.tensor_mul(zw3, G3, z3)
                dnum = sm.tile([128, T], F32, tag="dnum")
                nc.vector.tensor_reduce(
                    dnum[:], zw3, axis=mybir.AxisListType.X, op=ALU.add
                )

                # wtot = 1 - F_47
                wtot = sm.tile([128, T], F32, tag="wtot")
                nc.scalar.activation(
                    wtot[:].rearrange("p (t o) -> p t o", t=T),
                    F3[:, :, S - 1 : S],
                    ACTF.Copy,
                    bias=1.0,
                    scale=-1.0,
                )
                rec = sm.tile([128, T], F32, tag="rec")
                nc.vector.reciprocal(rec[:], wtot[:])

                # depth = clip(0.5 * dnum * rec, dmin, dmax)
                dc = outp.tile([128, T], F32, tag="dc")
                nc.vector.scalar_tensor_tensor(
                    dc[:], dnum[:], 0.5, rec[:], op0=ALU.mult, op1=ALU.mult
                )
                nc.vector.tensor_scalar(
                    dob[:, j * T : (j + 1) * T], dc[:], dmin, dmax,
                    op0=ALU.max, op1=ALU.min,
                )
                if j == DB - 1:
                    nc.sync.dma_start(
                        odep[rb].rearrange("(j p t) o -> p j t o", j=DB, p=128),
                        dob[:].rearrange("p (j t) -> p j t", j=DB).unsqueeze(3),
                    )

    nc.compile()
    return nc


def kernel(colors, densities, depths):
    colors = np.ascontiguousarray(colors, dtype=np.float32)
    densities = np.ascontiguousarray(densities, dtype=np.float32)
    depths = np.ascontiguousarray(depths, dtype=np.float32)

    cf = colors.reshape(NRAYS, S, C)
    df = densities.reshape(NRAYS, S)
    zf = depths.reshape(NRAYS, S)

    dmin = np.float32(zf.min())
    dmax = np.float32(zf.max())
    clip = np.tile(np.array([[dmin, dmax]], dtype=np.float32), (128, 1))

    if "nc" not in _CACHE:
        _CACHE["nc"] = _build_module()
    nc = _CACHE["nc"]

    ident = np.eye(128, dtype=np.float32)
    comb = np.tile(np.eye(C, dtype=np.float32), (128 // C, 1))
    in_maps = [
        {
            "colors": cf[k * PER : (k + 1) * PER],
            "densities": df[k * PER : (k + 1) * PER],
            "depths": zf[k * PER : (k + 1) * PER],
            "clipb": clip,
            "identb": ident,
            "combb": comb,
        }
        for k in range(NCORES)
    ]
    res = run_bass_kernel_spmd(nc, in_maps, core_ids=list(range(NCORES)))

    rgb = np.concatenate([r["orgb"] for r in res.results], axis=0)
    dep = np.concatenate([r["odep"] for r in res.results], axis=0)
    w = np.concatenate([r["ow"] for r in res.results], axis=0)

    return (
        rgb.reshape(B, R, C),
        dep.reshape(B, R, 1),
        w.reshape(B, R, S - 1, 1),
    )


if __name__ == "__main__":
    rng = np.random.default_rng(0)
    ins = {
        "colors": rng.random((B, R, S, C), dtype=np.float32),
        "densities": rng.standard_normal((B, R, S, 1)).astype(np.float32),
        "depths": np.sort(
            (rng.random((B, R, S, 1), dtype=np.float32) * 2.0 + 2.0), axis=2
        ),
    }
    outs = kernel(**ins)
    for o in outs:
        print(o.shape, o.dtype)


# revision 34
# speedup vs baseline: 1.0289x; 1.0289x over previous
"""Trainium2 Bass kernel for a NeRF-style mip ray marcher (alpha compositing).

Math (per ray, S=48 samples, C=32 channels):
    dd_i     = softplus(0.5*(den_i+den_{i+1}) - 1) * (dep_{i+1} - dep_i)   i=0..46
    F_k      = exp(-sum_{j<k} dd_j)              k=0..47   (F_0 = 1)
    w_i      = F_i - F_{i+1}                     (compositing weights)
    rgb_c    = sum_s G_s * col_{s,c} - 1         where G_s = F_{s-1} - F_{s+1}
               (G_s = w_{s-1}+w_s = 2*midpoint weight; folds the *2-1 output map)
    depth    = clip(0.5 * (sum_s G_s dep_s) / (1 - F_47), dmin, dmax)

Sharding: 65536 rays split evenly across 8 NeuronCores (SPMD, no comms).
"""

import sys

sys.path.insert(0, "/opt/trn_rl_repo")

import numpy as np

import concourse.bass as bass
import concourse.bacc as bacc
import concourse.mybir as mybir
import concourse.tile as tile
from concourse.bass_utils import run_bass_kernel_spmd

B, R, S, C = 4, 16384, 48, 32
NCORES = 8
NRAYS = B * R
PER = NRAYS // NCORES      # 8192 rays per core
T = 4                      # rays per partition per tile
RPT = 128 * T              # 512 rays per tile
NT = PER // RPT            # 16 tiles per core

F32 = mybir.dt.float32
ALU = mybir.AluOpType
ACTF = mybir.ActivationFunctionType

_CACHE = {}


def _pin_act_tables():
    """Restrict Exp/Ln/Copy to the one table set containing all three, so the
    table-load placement pass never alternates sets (each switch costs ~2.7us
    on hardware). Set ids are positional, so membership is edited in place
    rather than filtering entries."""
    import concourse.bacc as _bacc

    orig = _bacc.get_activation_tables
    keep = {ACTF.Exp, ACTF.Ln}

    def patched(arch):
        tabs = orig(arch)
        for name, s in tabs.items():
            if name != "natural_log_exp_and_others":
                s -= keep
        return tabs

    _bacc.get_activation_tables = patched


_pin_act_tables()

# op-placement / tiling configuration (tuned via timeline-sim sweeps)
CFG = {
    "T": 4,        # rays per partition per tile
    "small": "gpsimd",   # engine for small per-ray elementwise ops
    "scan": "vector",    # engine for the cumsum scan
    "tree": 0,     # levels of gpsimd pair-add reduction on wc before DVE reduce
    "cbufs": 3,    # colors pool buffers
    "wcbufs": 2,   # wc pool buffers
    "rgb": "pe",   # "dve": strided tensor_reduce; "pe": transpose+comb matmul
    "ndvecopy": 2, # with rgb=pe: how many of the 12 PSUM->SBUF copies go to DVE
    "trdtype": "f32",  # transpose input dtype view: "f32" or "f32r"
    "NG": 1,       # sample-windows per PSUM->SBUF copy group
    "trbufs": 4,   # PSUM transpose-staging buffers
    "smbufs": 4,   # small pool buffers
    "dbatch": 4,   # tiles per dens/deps input DMA and depth-out DMA
    "multsplit": 2,  # split wc multiply into this many DVE ops
    "outbufs": 4,  # output pool buffers
    "mmswap": 2,   # 2: interleaved fp32-exact; 1: t-major; 0: comb-stationary f32r
}


def _build_module(reps=1, cfg=CFG):
    global T, RPT, NT
    T = cfg["T"]
    RPT = 128 * T
    NT = PER // RPT
    nc = bacc.Bacc("TRN2", target_bir_lowering=False, debug=False, num_devices=NCORES)

    colors = nc.dram_tensor("colors", [PER, S, C], F32, kind="ExternalInput").ap()
    dens = nc.dram_tensor("densities", [PER, S], F32, kind="ExternalInput").ap()
    deps = nc.dram_tensor("depths", [PER, S], F32, kind="ExternalInput").ap()
    clipb = nc.dram_tensor("clipb", [128, 2], F32, kind="ExternalInput").ap()
    identb = nc.dram_tensor("identb", [128, 128], F32, kind="ExternalInput").ap()
    combb = nc.dram_tensor("combb", [128, C], mybir.dt.float32r, kind="ExternalInput").ap()
    combfb = nc.dram_tensor("combfb", [128, C], F32, kind="ExternalInput").ap()

    orgb = nc.dram_tensor("orgb", [PER, C], F32, kind="ExternalOutput").ap()
    odep = nc.dram_tensor("odep", [PER, 1], F32, kind="ExternalOutput").ap()
    ow = nc.dram_tensor("ow", [PER, S - 1], F32, kind="ExternalOutput").ap()

    smallE = getattr(nc, cfg["small"])
    scanE = getattr(nc, cfg["scan"])

    with tile.TileContext(nc) as tc:
        with (
            tc.tile_pool(name="const", bufs=1) as constp,
            tc.tile_pool(name="colors", bufs=cfg["cbufs"]) as cpool,
            tc.tile_pool(name="wc", bufs=cfg["wcbufs"]) as wcpool,
            tc.tile_pool(name="sm", bufs=cfg["smbufs"]) as sm,
            tc.tile_pool(name="outp", bufs=cfg["outbufs"]) as outp,
            tc.tile_pool(name="pstr", bufs=cfg["trbufs"], space="PSUM") as pstr,
            tc.tile_pool(name="psacc", bufs=(1 if cfg["mmswap"] == 2 else 2), space="PSUM") as psacc,
            tc.tile_pool(name="pstrb", bufs=2, space="PSUM") as pstrb,
        ):
            # constants
            clipt = constp.tile([128, 2], F32)
            nc.sync.dma_start(clipt[:], clipb)
            dmin = clipt[:, 0:1]
            dmax = clipt[:, 1:2]

            mask = constp.tile([128, T * S], F32)
            nc.gpsimd.memset(mask[:], 1.0)
            nc.gpsimd.memset(
                mask[:].rearrange("p (t s) -> p t s", t=T)[:, :, 0:1], 0.0
            )

            neg1 = constp.tile([128, 1], F32)
            nc.gpsimd.memset(neg1[:], -1.0)

            if cfg["rgb"] == "pe":
                ident = constp.tile([128, 128], F32)
                nc.sync.dma_start(ident[:], identb)
                comb = constp.tile([128, C], mybir.dt.float32r)
                nc.sync.dma_start(comb[:], combb)
                combf = constp.tile([128, C], F32)
                nc.sync.dma_start(combf[:], combfb)

            for i in range(NT * reps):
                i = i % NT
                r0 = i * RPT
                rs = slice(r0, r0 + RPT)

                ct = cpool.tile([128, T * S * C], F32)
                ct4 = ct[:].rearrange("p (t s c) -> p t s c", t=T, s=S)
                nc.sync.dma_start(
                    ct4, colors[rs].rearrange("(p t) s c -> p t s c", p=128)
                )

                DB = cfg["dbatch"]
                if i % DB == 0:
                    rb = slice(r0, r0 + RPT * DB)
                    dtb = sm.tile([128, DB * T * S], F32, tag="dens")
                    nc.scalar.dma_start(
                        dtb[:].rearrange("p (j t s) -> p j t s", j=DB, t=T),
                        dens[rb].rearrange("(j p t) s -> p j t s", j=DB, p=128),
                    )
                    ztb = sm.tile([128, DB * T * S], F32, tag="deps")
                    nc.scalar.dma_start(
                        ztb[:].rearrange("p (j t s) -> p j t s", j=DB, t=T),
                        deps[rb].rearrange("(j p t) s -> p j t s", j=DB, p=128),
                    )
                    dob = outp.tile([128, DB * T], F32, tag="dob")
                j = i % DB
                d3 = dtb[:, j * T * S : (j + 1) * T * S].rearrange(
                    "p (t s) -> p t s", t=T
                )
                z3 = ztb[:, j * T * S : (j + 1) * T * S].rearrange(
                    "p (t s) -> p t s", t=T
                )

                # densities midpoint sum (pre-affine) [128, T, 47]
                pd = sm.tile([128, T * (S - 1)], F32, tag="pd")
                pd3 = pd[:].rearrange("p (t s) -> p t s", t=T)
                smallE.tensor_add(pd3, d3[:, :, 0 : S - 1], d3[:, :, 1:S])

                # dm = softplus(0.5*pd - 1) = ln(1 + exp(0.5*pd - 1))
                es = sm.tile([128, T * (S - 1)], F32, tag="es")
                es3 = es[:].rearrange("p (t s) -> p t s", t=T)
                nc.scalar.activation(es3, pd3, ACTF.Exp, bias=neg1[:], scale=0.5)
                dm = sm.tile([128, T * (S - 1)], F32, tag="dm")
                dm3 = dm[:].rearrange("p (t s) -> p t s", t=T)
                nc.scalar.activation(dm3, es3, ACTF.Ln, bias=1.0, scale=1.0)

                # delta = dep[s+1] - dep[s]
                dl = sm.tile([128, T * (S - 1)], F32, tag="delta")
                dl3 = dl[:].rearrange("p (t s) -> p t s", t=T)
                smallE.tensor_sub(dl3, z3[:, :, 1:S], z3[:, :, 0 : S - 1])

                # e-buffer: slot 0 = 0, slots 1..47 = dd
                eb = sm.tile([128, T * S], F32, tag="ebuf")
                e3 = eb[:].rearrange("p (t s) -> p t s", t=T)
                nc.gpsimd.memset(e3[:, :, 0:1], 0.0)
                smallE.tensor_mul(e3[:, :, 1:S], dm3, dl3)

                # exclusive cumsum of dd via masked scan (mask resets at ray start)
                Dt = sm.tile([128, T * S], F32, tag="Dt")
                scanE.tensor_tensor_scan(
                    Dt[:], mask[:], eb[:], 0.0, op0=ALU.mult, op1=ALU.add
                )

                # F = exp(-D)
                Ft = sm.tile([128, T * S], F32, tag="Ft")
                F3 = Ft[:].rearrange("p (t s) -> p t s", t=T)
                nc.scalar.activation(Ft[:], Dt[:], ACTF.Exp, scale=-1.0)

                # weights w_i = F_i - F_{i+1}
                wt = outp.tile([128, T * (S - 1)], F32, tag="w")
                w3 = wt[:].rearrange("p (t s) -> p t s", t=T)
                smallE.tensor_sub(w3, F3[:, :, 0 : S - 1], F3[:, :, 1:S])
                nc.scalar.dma_start(ow[rs].rearrange("(p t) s -> p t s", p=128), w3)

                # G_s = F_{s-1} - F_{s+1}; edges G_0 = w_0, G_47 = w_46
                Gt = sm.tile([128, T * S], F32, tag="Gt")
                G3 = Gt[:].rearrange("p (t s) -> p t s", t=T)
                smallE.tensor_sub(
                    G3[:, :, 1 : S - 1], F3[:, :, 0 : S - 2], F3[:, :, 2:S]
                )
                nc.gpsimd.tensor_copy(G3[:, :, 0:1], w3[:, :, 0:1])
                nc.gpsimd.tensor_copy(G3[:, :, S - 1 : S], w3[:, :, S - 2 : S - 1])

                # wc = colors * G (broadcast over channels)
                wc = wcpool.tile([128, T * S * C], F32)
                wc4 = wc[:].rearrange("p (t s c) -> p t s c", t=T, s=S)
                Gb = (
                    Gt[:]
                    .rearrange("p (t s) -> p t s", t=T)
                    .unsqueeze(3)
                    .broadcast_to((128, T, S, C))
                )
                ms = cfg["multsplit"]
                step = S // ms
                for m in range(ms):
                    sl = slice(m * step, (m + 1) * step)
                    nc.vector.tensor_mul(wc4[:, :, sl, :], ct4[:, :, sl, :], Gb[:, :, sl, :])

                # rgb = sum_s wc - 1
                rgbo = outp.tile([128, T * C], F32, tag="rgbo")
                rgbo3 = rgbo[:].rearrange("p (t c) -> p t c", t=T)
                if cfg["rgb"] == "dve":
                    red_in = wc
                    sred = S
                    for lvl in range(cfg["tree"]):
                        half = sred // 2
                        h = wcpool.tile([128, T * half * C], F32, tag=f"h{lvl}")
                        h4 = h[:].rearrange("p (t s c) -> p t s c", t=T, s=half)
                        r4 = red_in[:].rearrange("p (t s c) -> p t s c", t=T, s=sred)
                        nc.gpsimd.tensor_add(h4, r4[:, :, 0:half, :], r4[:, :, half:sred, :])
                        red_in = h
                        sred = half
                    rgbs = sm.tile([128, T * C], F32, tag="rgbs")
                    rgbs3 = rgbs[:].rearrange("p (t c) -> p t c", t=T)
                    nc.vector.tensor_reduce(
                        rgbs3,
                        red_in[:].rearrange("p (t s c) -> p t c s", t=T, s=sred),
                        axis=mybir.AxisListType.X,
                        op=ALU.add,
                    )
                    nc.scalar.activation(rgbo[:], rgbs[:], ACTF.Copy, bias=-1.0)
                else:
                    # PE path: transpose 128x128 blocks of wc into PSUM so the
                    # sample axis lands on partitions, copy to SBUF, then a
                    # comb matmul (one-hot channel selector) contracts the 4
                    # samples of each block; PSUM-accumulate over the 12
                    # sample-windows. acc[c', t*128+p] = sum_s G*col.
                    NW = S // 4                # 12 sample-windows per t
                    NG = cfg["NG"]             # windows per PSUM copy group
                    f32r = mybir.dt.float32r
                    if cfg["mmswap"] == 2:
                        # interleaved windows, one PSUM bank per t (separate
                        # tags so start=True only resets its own bank), full
                        # fp32 matmuls with wcT stationary / comb moving (N=32)
                        # -> exact fp32 precision, rgb lands in ray layout.
                        acc4 = []
                        for t in range(T):
                            a_ = psacc.tile([128, C], F32, tag=f"acc{t}", name=f"acc{t}")
                            acc4.append(a_)
                        for og in range(NW // NG):
                            trp = pstr.tile([128, 128 * T * NG], F32, tag="tr")
                            for g in range(NG):
                                o = og * NG + g
                                for t in range(T):
                                    blk = wc[
                                        :,
                                        t * S * C + o * 128 : t * S * C + (o + 1) * 128,
                                    ]
                                    nc.tensor.transpose(
                                        trp[:, (g * T + t) * 128 : (g * T + t + 1) * 128],
                                        blk,
                                        ident[:],
                                    )
                            wcsb = sm.tile([128, 128 * T * NG], F32, tag="wcsb")
                            if og < cfg["ndvecopy"]:
                                nc.vector.tensor_copy(wcsb[:], trp[:])
                            else:
                                nc.scalar.copy(wcsb[:], trp[:])
                            for g in range(NG):
                                o = og * NG + g
                                for t in range(T):
                                    nc.tensor.matmul(
                                        acc4[t][:],
                                        wcsb[:, (g * T + t) * 128 : (g * T + t + 1) * 128],
                                        combf[:],
                                        start=(o == 0),
                                        stop=(o == NW - 1),
                                    )
                        for t in range(T):
                            nc.scalar.activation(
                                rgbo3[:, t, :], acc4[t][:], ACTF.Copy, bias=-1.0
                            )
                    elif cfg["mmswap"]:
                        # t-major: each t accumulates into its own PSUM bank
                        # (pool-rotated), so matmul start=True never clobbers
                        # another t's partial sums. rgb lands in ray layout.
                        for t in range(T):
                            acc = psacc.tile([128, C], F32, tag="acc")
                            for og in range(NW // NG):
                                trp = pstr.tile([128, 128 * NG], F32, tag="tr")
                                for g in range(NG):
                                    o = og * NG + g
                                    blk = wc[
                                        :,
                                        t * S * C + o * 128 : t * S * C + (o + 1) * 128,
                                    ]
                                    idap = ident[:]
                                    if cfg["trdtype"] == "f32r":
                                        blk = blk.bitcast(f32r)
                                        idap = idap.bitcast(f32r)
                                    nc.tensor.transpose(
                                        trp[:, g * 128 : (g + 1) * 128], blk, idap
                                    )
                                wcsb = sm.tile(
                                    [128, 128 * NG], mybir.dt.float32r, tag="wcsb"
                                )
                                if og < cfg["ndvecopy"]:
                                    nc.vector.tensor_copy(wcsb[:], trp[:])
                                else:
                                    nc.scalar.copy(wcsb[:], trp[:])
                                for g in range(NG):
                                    o = og * NG + g
                                    nc.tensor.matmul(
                                        acc[:],
                                        wcsb[:, g * 128 : (g + 1) * 128],
                                        comb[:],
                                        start=(o == 0),
                                        stop=(o == NW - 1),
                                    )
                            nc.scalar.activation(
                                rgbo3[:, t, :], acc[:], ACTF.Copy, bias=-1.0
                            )
                    else:
                        acc = psacc.tile([C, 128 * T], F32, tag="acc")
                        for og in range(NW // NG):
                            trp = pstr.tile([128, 128 * T * NG], F32, tag="tr")
                            for g in range(NG):
                                o = og * NG + g
                                for t in range(T):
                                    blk = wc[
                                        :,
                                        t * S * C + o * 128 : t * S * C + (o + 1) * 128,
                                    ]
                                    idap = ident[:]
                                    if cfg["trdtype"] == "f32r":
                                        blk = blk.bitcast(f32r)
                                        idap = idap.bitcast(f32r)
                                    nc.tensor.transpose(
                                        trp[:, (g * T + t) * 128 : (g * T + t + 1) * 128],
                                        blk,
                                        idap,
                                    )
                            wcsb = sm.tile(
                                [128, 128 * T * NG], mybir.dt.float32r, tag="wcsb"
                            )
                            if og < cfg["ndvecopy"]:
                                nc.vector.tensor_copy(wcsb[:], trp[:])
                            else:
                                nc.scalar.copy(wcsb[:], trp[:])
                            for g in range(NG):
                                o = og * NG + g
                                nc.tensor.matmul(
                                    acc[:],
                                    comb[:],
                                    wcsb[:, g * 128 * T : (g + 1) * 128 * T],
                                    start=(o == 0),
                                    stop=(o == NW - 1),
                                )
                        accsb = sm.tile([C, 128 * T], F32, tag="accs")
                        nc.scalar.copy(accsb[:], acc[:])
                        for t in range(T):
                            trb = pstrb.tile([128, C], F32, tag="trb")
                            nc.tensor.transpose(
                                trb[:], accsb[:, t * 128 : (t + 1) * 128], ident[:C, :C]
                            )
                            nc.scalar.activation(
                                rgbo3[:, t, :], trb[:], ACTF.Copy, bias=-1.0
                            )
                nc.sync.dma_start(
                    orgb[rs].rearrange("(p t) c -> p t c", p=128), rgbo3
                )

                # depth numerator: sum_s G_s * dep_s
                zw = sm.tile([128, T * S], F32, tag="zw")
                zw3 = zw[:].rearrange("p (t s) -> p t s", t=T)
                smallE.tensor_mul(zw3, G3, z3)
                dnum = sm.tile([128, T], F32, tag="dnum")
                nc.vector.tensor_reduce(
                    dnum[:], zw3, axis=mybir.AxisListType.X, op=ALU.add
                )

                # wtot = 1 - F_47
                wtot = sm.tile([128, T], F32, tag="wtot")
                nc.scalar.activation(
                    wtot[:].rearrange("p (t o) -> p t o", t=T),
                    F3[:, :, S - 1 : S],
                    ACTF.Copy,
                    bias=1.0,
                    scale=-1.0,
                )
                rec = sm.tile([128, T], F32, tag="rec")
                nc.vector.reciprocal(rec[:], wtot[:])

                # depth = clip(0.5 * dnum * rec, dmin, dmax)
                dc = outp.tile([128, T], F32, tag="dc")
                nc.vector.scalar_tensor_tensor(
                    dc[:], dnum[:], 0.5, rec[:], op0=ALU.mult, op1=ALU.mult
                )
                nc.vector.tensor_scalar(
                    dob[:, j * T : (j + 1) * T], dc[:], dmin, dmax,
                    op0=ALU.max, op1=ALU.min,
                )
                if j == DB - 1:
                    nc.sync.dma_start(
                        odep[rb].rearrange("(j p t) o -> p j t o", j=DB, p=128),
                        dob[:].rearrange("p (j t) -> p j t", j=DB).unsqueeze(3),
                    )

    nc.compile()
    return nc


def kernel(colors, densities, depths):
    colors = np.ascontiguousarray(colors, dtype=np.float32)
    densities = np.ascontiguousarray(densities, dtype=np.float32)
    depths = np.ascontiguousarray(depths, dtype=np.float32)

    cf = colors.reshape(NRAYS, S, C)
    df = densities.reshape(NRAYS, S)
    zf = depths.reshape(NRAYS, S)

    dmin = np.float32(zf.min())
    dmax = np.float32(zf.max())
    clip = np.tile(np.array([[dmin, dmax]], dtype=np.float32), (128, 1))

    if "nc" not in _CACHE:
        _CACHE["nc"] = _build_module()
    nc = _CACHE["nc"]

    ident = np.eye(128, dtype=np.float32)
    comb = np.tile(np.eye(C, dtype=np.float32), (128 // C, 1))
    in_maps = [
        {
            "colors": cf[k * PER : (k + 1) * PER],
            "densities": df[k * PER : (k + 1) * PER],
            "depths": zf[k * PER : (k + 1) * PER],
            "clipb": clip,
            "identb": ident,
            "combb": comb,
            "combfb": comb,
        }
        for k in range(NCORES)
    ]
    res = run_bass_kernel_spmd(nc, in_maps, core_ids=list(range(NCORES)))

    rgb = np.concatenate([r["orgb"] for r in res.results], axis=0)
    dep = np.concatenate([r["odep"] for r in res.results], axis=0)
    w = np.concatenate([r["ow"] for r in res.results], axis=0)

    return (
        rgb.reshape(B, R, C),
        dep.reshape(B, R, 1),
        w.reshape(B, R, S - 1, 1),
    )


if __name__ == "__main__":
    rng = np.random.default_rng(0)
    ins = {
        "colors": rng.random((B, R, S, C), dtype=np.float32),
        "densities": rng.standard_normal((B, R, S, 1)).astype(np.float32),
        "depths": np.sort(
            (rng.random((B, R, S, 1), dtype=np.float32) * 2.0 + 2.0), axis=2
        ),
    }
    outs = kernel(**ins)
    for o in outs:
        print(o.shape, o.dtype)
